# revision 56
# baseline (speedup 1.0000x reference)
"""HGATConv on 8 trn2 NeuronCores via Bass/Tile.

Math (equivalent to reference; softmax without max-shift — logits are small):
  h = x@W + b;  a_n = h@attn_node;  e = exp(a_n)
  stage1: hhat[j] = sum_{i: he_i=j} e[n_i]*h[n_i];  q[j] = sum_{i: he_i=j} e[n_i]
          S1 = sum_i e[n_i] = sum_n cnt[n]*e[n]
  a_e = (hhat @ attn_edge)/S1;  u = exp(a_e);  T2[j] = (u[j]/S1)*hhat[j]
  S2 = sum_j u[j]*q[j]
  h_n[n] = (e[n]/S2) * sum_{i: node_i=n} T2[he_i]

Three SPMD launches. The host performs the halo exchange between launches
(pure data movement: fancy-indexed row routing of per-incidence payloads,
sorted by destination block and padded to core-uniform chunk counts) so
every device-side DMA is a large streaming HWDGE transfer — no dma_gather.
Per-incidence payloads travel as bf16 (tolerance is 2e-2).

  A: node shard -> bf16 row table [e*(x@W) | e | pad] (132 cols), e table,
     S1 partial. One bf16 matmul per 128-node block computes [h | a_n] via
     extended weights [W | W@attn_node]; bias b is folded into launch B
     exactly via hhat_corrected = hhat + q*b.
  B: stream stage-1 rows; segment-sum via bf16 selection matmuls. Incidences
     are routed per (dest block, 64-dest half) so sel is [128, 64] (half the
     DVE is_equal work) and each matmul writes its half of the block's PSUM.
     Per-block epilogue computes u, the T2 row block, and the S2 partial.
  C: stream stage-2 T2 rows; same half-split selection matmuls; scale by
     e[n]/S2 -> h_n (bf16, host widens to fp32 losslessly)
"""
import os
import sys

sys.path.insert(0, os.path.dirname(os.path.abspath(__file__)))
try:
    import ntff_shim  # noqa: F401  (optional; enables trace under axon)
except Exception:
    pass

import numpy as np
import ml_dtypes
import concourse.bacc as bacc
import concourse.mybir as mybir
import concourse.tile as tile
from concourse.bass_utils import run_bass_kernel_spmd

f32 = mybir.dt.float32
bf16 = mybir.dt.bfloat16
BF = ml_dtypes.bfloat16
P = 128
NC = 8
N, H, M, D = 100000, 20000, 600000, 128
NSH, HSH = N // NC, H // NC          # 12500, 2500
NBA = (NSH + P - 1) // P             # 98 node blocks per core
NSHP = NBA * P                       # 12544
NBB = (HSH + P - 1) // P             # 20 hyperedge blocks per core
EW = 132                             # stage-1 row floats: [e*h(128) | e | 3 pad]
KMAX = 16                            # sel-matrix chunks per DVE op
CGRP = 16                            # stage-2 blocks per rows DMA

LAST_EXEC_TIMES = []
_TRACE = bool(os.environ.get("HGAT_TRACE"))

Alu = mybir.AluOpType
Act = mybir.ActivationFunctionType


def _run(nc, ins, tag):
    nc.finalize()
    res = run_bass_kernel_spmd(nc, ins, list(range(NC)), trace=_TRACE)
    if _TRACE:
        LAST_EXEC_TIMES.append((tag, res.exec_time_ns, res.mean_exec_time_ns))
    return res.results


# ---------------------------------------------------------------- launch A
def _build_launch_a():
    nc = bacc.Bacc("TRN2")
    xT = nc.declare_dram_parameter("xT", [P, NSHP], f32, isOutput=False)
    Wp = nc.declare_dram_parameter("W", [P, D], bf16, isOutput=False)
    WT = nc.declare_dram_parameter("WT", [P, D], bf16, isOutput=False)
    b_col = nc.declare_dram_parameter("b_col", [P, 1], bf16, isOutput=False)
    an_col = nc.declare_dram_parameter("an_col", [P, 1], bf16, isOutput=False)
    ones_row = nc.declare_dram_parameter("ones_row", [1, P], bf16, isOutput=False)
    ones_col = nc.declare_dram_parameter("ones_col", [P, 1], bf16, isOutput=False)
    cnt_w = nc.declare_dram_parameter("cnt_w", [P, NBA], f32, isOutput=False)
    g_sh = nc.declare_dram_parameter("g_sh", [P, NBA * EW], bf16, isOutput=True)
    exan_sh = nc.declare_dram_parameter("exan_sh", [P, NBA], f32, isOutput=True)
    s1_part = nc.declare_dram_parameter("s1_part", [1, 1], f32, isOutput=True)

    XSEC = [0, 25, 50, 75, NBA]  # x section boundaries (blocks)
    with tile.TileContext(nc) as tc:
        with (
            tc.tile_pool(name="sbuf", bufs=1) as pool,
            tc.tile_pool(name="work", bufs=2) as wpool,
            tc.tile_pool(name="psh", bufs=6, space="PSUM") as pph,
            tc.tile_pool(name="pscl", bufs=1, space="PSUM") as pscl,
        ):
            wbf = pool.tile([P, D], bf16)
            nc.sync.dma_start(out=wbf[:], in_=Wp[:])
            wtt = pool.tile([P, D], bf16)
            nc.sync.dma_start(out=wtt[:], in_=WT[:])
            bcol_t = pool.tile([P, 1], bf16)
            nc.sync.dma_start(out=bcol_t[:], in_=b_col[:])
            ancol_t = pool.tile([P, 1], bf16)
            nc.sync.dma_start(out=ancol_t[:], in_=an_col[:])
            onr_bf = pool.tile([1, P], bf16)
            nc.sync.dma_start(out=onr_bf[:], in_=ones_row[:])
            onc_bf = pool.tile([P, 1], bf16)
            nc.sync.dma_start(out=onc_bf[:], in_=ones_col[:])
            cnt_t = pool.tile([P, NBA], f32)
            nc.sync.dma_start(out=cnt_t[:], in_=cnt_w[:])
            xsec = []
            for s in range(4):
                c0, c1 = XSEC[s] * P, XSEC[s + 1] * P
                xs_t = pool.tile([P, c1 - c0], f32, name=f"xs{s}")
                nc.sync.dma_start(out=xs_t[:], in_=xT[:, c0:c1])
                xb_t = pool.tile([P, c1 - c0], bf16, name=f"xb{s}")
                nc.vector.tensor_copy(out=xb_t[:], in_=xs_t[:])
                xsec.append(xb_t)

            # wa = W @ attn_node (via W^T as lhsT); ab = b @ attn_node
            # wext = [W | wa] so h and a_n come from ONE matmul per block
            pw = pscl.tile([P, 1], f32, tag="scl", space="PSUM")
            nc.tensor.matmul(out=pw[:], lhsT=wtt[:], rhs=ancol_t[:], start=True, stop=True)
            wext = pool.tile([P, D + 1], bf16)
            nc.vector.tensor_copy(out=wext[:, 0:D], in_=wbf[:])
            nc.vector.tensor_copy(out=wext[:, D : D + 1], in_=pw[:])
            pab = pscl.tile([1, 1], f32, tag="scl", space="PSUM")
            nc.tensor.matmul(out=pab[:], lhsT=bcol_t[:], rhs=ancol_t[:], start=True, stop=True)
            ab_sb = pool.tile([1, 1], bf16)
            nc.vector.tensor_copy(out=ab_sb[:], in_=pab[:])
            pabc = pscl.tile([P, 1], f32, tag="scl", space="PSUM")
            nc.tensor.matmul(out=pabc[:], lhsT=onr_bf[:], rhs=ab_sb[:], start=True, stop=True)
            ab_col = pool.tile([P, 1], f32)
            nc.vector.tensor_copy(out=ab_col[:], in_=pabc[:])

            gbig0 = pool.tile([P, 49 * EW], bf16)
            gbig1 = pool.tile([P, (NBA - 49) * EW], bf16)
            nc.gpsimd.memset(gbig0[:], 0)
            nc.gpsimd.memset(gbig1[:], 0)
            exan = pool.tile([P, NBA], f32)

            for t in range(NBA):
                s = min(3, t // 25)
                xs = xsec[s][:, (t - XSEC[s]) * P : (t - XSEC[s] + 1) * P]
                gb = gbig0 if t < 49 else gbig1
                go = (t if t < 49 else t - 49) * EW
                # one matmul per block: [h | a_n] = x @ [W | wa]
                ps_h = pph.tile([P, D + 1], f32, tag="ph", space="PSUM")
                nc.tensor.matmul(out=ps_h[:], lhsT=xs, rhs=wext[:], start=True, stop=True)
                ecol = exan[:, t : t + 1]
                nc.scalar.activation(
                    out=ecol, in_=ps_h[:, D : D + 1], func=Act.Exp, bias=ab_col[:]
                )
                if t % 2 == 0:
                    # balance the e*h scale between DVE and scalar engines
                    nc.vector.tensor_scalar(
                        out=gb[:, go : go + D], in0=ps_h[:, 0:D], scalar1=ecol,
                        scalar2=None, op0=Alu.mult,
                    )
                else:
                    nc.scalar.activation(
                        out=gb[:, go : go + D], in_=ps_h[:, 0:D], func=Act.Copy,
                        scale=ecol,
                    )
                nc.vector.tensor_copy(out=gb[:, go + D : go + D + 1], in_=ecol)
                if t == 48:
                    nc.sync.dma_start(out=g_sh[:, : 49 * EW], in_=gbig0[:])
            nc.sync.dma_start(out=g_sh[:, 49 * EW :], in_=gbig1[:])
            nc.sync.dma_start(out=exan_sh[:], in_=exan[:])

            # S1 partial = sum(cnt * e) over this core's shard
            scr = wpool.tile([P, NBA], f32, tag="scr")
            s1col = pool.tile([P, 1], f32)
            nc.vector.tensor_tensor(
                out=scr[:], in0=exan[:], in1=cnt_t[:], op=Alu.mult
            )
            nc.vector.tensor_reduce(
                out=s1col[:], in_=scr[:], axis=mybir.AxisListType.X, op=Alu.add
            )
            s1bf = pool.tile([P, 1], bf16)
            nc.vector.tensor_copy(out=s1bf[:], in_=s1col[:])
            ps1 = pscl.tile([1, 1], f32, tag="scl", space="PSUM")
            nc.tensor.matmul(out=ps1[:], lhsT=s1bf[:], rhs=onc_bf[:], start=True, stop=True)
            s1sb = pool.tile([1, 1], f32)
            nc.vector.tensor_copy(out=s1sb[:], in_=ps1[:])
            nc.sync.dma_start(out=s1_part[:], in_=s1sb[:])
    return nc


# ---------------------------------------------------------------- launch B
def _build_launch_b(cb1):
    """cb1: chunks per (block, half) — length 2*NBB, order (b0,h0),(b0,h1),..."""
    TOT1 = int(sum(cb1))
    CBMAX = max(int(cb1[2 * b] + cb1[2 * b + 1]) for b in range(NBB))
    HD = P // 2
    nc = bacc.Bacc("TRN2")
    rows = nc.declare_dram_parameter("rows", [P, TOT1 * EW], bf16, isOutput=False)
    rel = nc.declare_dram_parameter("rel", [P, TOT1], bf16, isOutput=False)
    iota = nc.declare_dram_parameter("iota", [P, KMAX * HD], bf16, isOutput=False)
    ae_bc = nc.declare_dram_parameter("ae_bc", [P, D], f32, isOutput=False)
    b_bc = nc.declare_dram_parameter("b_bc", [P, D], f32, isOutput=False)
    s1p = nc.declare_dram_parameter("s1p", [P, NC], f32, isOutput=False)
    ones_col = nc.declare_dram_parameter("ones_col", [P, 1], bf16, isOutput=False)
    t2o = nc.declare_dram_parameter("t2o", [P, NBB * D], bf16, isOutput=True)
    s2_part = nc.declare_dram_parameter("s2_part", [1, 1], f32, isOutput=True)

    with tile.TileContext(nc) as tc:
        with (
            tc.tile_pool(name="sbuf", bufs=1) as pool,
            tc.tile_pool(name="rows", bufs=4) as rpool,
            tc.tile_pool(name="sel", bufs=4) as spool,
            tc.tile_pool(name="work", bufs=2) as wpool,
            tc.tile_pool(name="psum", bufs=2, space="PSUM") as pp,
            tc.tile_pool(name="pscl", bufs=1, space="PSUM") as pscl,
        ):
            rel_t = pool.tile([P, TOT1], bf16)
            nc.sync.dma_start(out=rel_t[:], in_=rel[:])
            iota_t = pool.tile([P, KMAX * HD], bf16)
            nc.sync.dma_start(out=iota_t[:], in_=iota[:])
            ae_t = pool.tile([P, D], f32)
            nc.sync.dma_start(out=ae_t[:], in_=ae_bc[:])
            bb_t = pool.tile([P, D], f32)
            nc.sync.dma_start(out=bb_t[:], in_=b_bc[:])
            s1p_t = pool.tile([P, NC], f32)
            nc.sync.dma_start(out=s1p_t[:], in_=s1p[:])
            onc_bf = pool.tile([P, 1], bf16)
            nc.sync.dma_start(out=onc_bf[:], in_=ones_col[:])

            s1tot = pool.tile([P, 1], f32)
            nc.vector.tensor_reduce(
                out=s1tot[:], in_=s1p_t[:], axis=mybir.AxisListType.X, op=Alu.add
            )
            rs1c = pool.tile([P, 1], f32)
            nc.vector.reciprocal(out=rs1c[:], in_=s1tot[:])

            t2big = pool.tile([P, NBB * D], bf16)
            s2acc = pool.tile([P, 1], f32)
            nc.vector.memset(s2acc[:], 0)

            off = 0
            for b in range(NBB):
                nb0, nb1 = int(cb1[2 * b]), int(cb1[2 * b + 1])
                nb = nb0 + nb1
                rt = rpool.tile([P, CBMAX * EW], bf16, tag="rows")
                nc.sync.dma_start(
                    out=rt[:, : nb * EW], in_=rows[:, off * EW : (off + nb) * EW]
                )
                ps = pp.tile([P, D + 1], f32, tag="ps", space="PSUM")
                for half, h0, nbh in ((0, 0, nb0), (1, nb0, nb1)):
                    ci = 0
                    for g0 in range(0, nbh, KMAX):
                        G = min(KMAX, nbh - g0)
                        sel = spool.tile([P, KMAX * HD], bf16, tag="sel")
                        nc.vector.tensor_tensor(
                            out=sel[:, : G * HD],
                            in0=iota_t[:, : G * HD],
                            in1=rel_t[
                                :, off + h0 + g0 : off + h0 + g0 + G
                            ].to_broadcast([P, G, HD]),
                            op=Alu.is_equal,
                        )
                        for j in range(G):
                            c = h0 + g0 + j
                            nc.tensor.matmul(
                                out=ps[half * HD : (half + 1) * HD, :],
                                lhsT=sel[:, j * HD : (j + 1) * HD],
                                rhs=rt[:, c * EW : c * EW + D + 1],
                                start=(ci == 0), stop=(ci == nbh - 1),
                            )
                            ci += 1
                # epilogue: hh = hhat + q*b, then a_e, u, T2 block, S2 partial
                hh = wpool.tile([P, D], f32, tag="hh")
                nc.vector.scalar_tensor_tensor(
                    out=hh[:], in0=bb_t[:], scalar=ps[:, D : D + 1], in1=ps[:, 0:D],
                    op0=Alu.mult, op1=Alu.add,
                )
                scr = wpool.tile([P, D], f32, tag="scr")
                araw = wpool.tile([P, 1], f32, tag="araw")
                nc.vector.tensor_tensor(
                    out=scr[:], in0=hh[:], in1=ae_t[:], op=Alu.mult
                )
                nc.vector.tensor_reduce(
                    out=araw[:], in_=scr[:], axis=mybir.AxisListType.X, op=Alu.add
                )
                ucol = wpool.tile([P, 1], f32, tag="ucol")
                nc.scalar.activation(out=ucol[:], in_=araw[:], func=Act.Exp, scale=rs1c[:])
                wcol = wpool.tile([P, 1], f32, tag="wcol")
                nc.vector.tensor_tensor(
                    out=wcol[:], in0=ucol[:], in1=rs1c[:], op=Alu.mult
                )
                nc.scalar.activation(
                    out=t2big[:, b * D : (b + 1) * D], in_=hh[:],
                    func=Act.Copy, scale=wcol[:],
                )
                nc.vector.scalar_tensor_tensor(
                    out=s2acc[:], in0=ucol[:], scalar=ps[:, D : D + 1], in1=s2acc[:],
                    op0=Alu.mult, op1=Alu.add,
                )
                off += nb

            nc.sync.dma_start(out=t2o[:], in_=t2big[:])
            s2bf = pool.tile([P, 1], bf16)
            nc.vector.tensor_copy(out=s2bf[:], in_=s2acc[:])
            ps2 = pscl.tile([1, 1], f32, tag="ps2", space="PSUM")
            nc.tensor.matmul(out=ps2[:], lhsT=s2bf[:], rhs=onc_bf[:], start=True, stop=True)
            s2sb = pool.tile([1, 1], f32)
            nc.vector.tensor_copy(out=s2sb[:], in_=ps2[:])
            nc.sync.dma_start(out=s2_part[:], in_=s2sb[:])
    return nc


# ---------------------------------------------------------------- launch C
def _build_launch_c(cb2):
    """cb2: chunks per (block, half) — length 2*NBA."""
    TOT2 = int(sum(cb2))
    HD = P // 2
    nblk = [int(cb2[2 * b] + cb2[2 * b + 1]) for b in range(NBA)]
    # rows DMA groups of CGRP blocks
    groups = []
    for g0 in range(0, NBA, CGRP):
        blks = list(range(g0, min(NBA, g0 + CGRP)))
        groups.append(blks)
    GMAX = max(sum(nblk[b] for b in blks) for blks in groups)
    HSEC = [0, 25, 50, 75, NBA]  # h_n output sections

    nc = bacc.Bacc("TRN2")
    rows = nc.declare_dram_parameter("rows", [P, TOT2 * D], bf16, isOutput=False)
    rel = nc.declare_dram_parameter("rel", [P, TOT2], bf16, isOutput=False)
    iota = nc.declare_dram_parameter("iota", [P, KMAX * HD], bf16, isOutput=False)
    exsh = nc.declare_dram_parameter("exsh", [P, NBA], f32, isOutput=False)
    s2p = nc.declare_dram_parameter("s2p", [P, NC], f32, isOutput=False)
    hno = nc.declare_dram_parameter("hno", [P, NBA * D], bf16, isOutput=True)

    with tile.TileContext(nc) as tc:
        with (
            tc.tile_pool(name="sbuf", bufs=1) as pool,
            tc.tile_pool(name="rows", bufs=4) as rpool,
            tc.tile_pool(name="sel", bufs=4) as spool,
            tc.tile_pool(name="work", bufs=2) as wpool,
            tc.tile_pool(name="hsec", bufs=2) as hpool,
            tc.tile_pool(name="psum", bufs=2, space="PSUM") as pp,
        ):
            rel_t = pool.tile([P, TOT2], bf16)
            nc.sync.dma_start(out=rel_t[:], in_=rel[:])
            iota_t = pool.tile([P, KMAX * HD], bf16)
            nc.sync.dma_start(out=iota_t[:], in_=iota[:])
            ex_t = pool.tile([P, NBA], f32)
            nc.sync.dma_start(out=ex_t[:], in_=exsh[:])
            s2p_t = pool.tile([P, NC], f32)
            nc.sync.dma_start(out=s2p_t[:], in_=s2p[:])

            s2tot = pool.tile([P, 1], f32)
            nc.vector.tensor_reduce(
                out=s2tot[:], in_=s2p_t[:], axis=mybir.AxisListType.X, op=Alu.add
            )
            rs2c = pool.tile([P, 1], f32)
            nc.vector.reciprocal(out=rs2c[:], in_=s2tot[:])

            hsec_t = None
            hs = 0
            off = 0
            for blks in groups:
                gtot = sum(nblk[b] for b in blks)
                rt = rpool.tile([P, GMAX * D], bf16, tag="rows")
                nc.sync.dma_start(
                    out=rt[:, : gtot * D], in_=rows[:, off * D : (off + gtot) * D]
                )
                loc = 0
                for b in blks:
                    nb0, nb1 = int(cb2[2 * b]), int(cb2[2 * b + 1])
                    if b == HSEC[hs]:
                        hsec_t = hpool.tile(
                            [P, (HSEC[hs + 1] - HSEC[hs]) * D], bf16, tag="hsec"
                        )
                    ps = pp.tile([P, D], f32, tag="ps", space="PSUM")
                    for half, h0, nbh in ((0, 0, nb0), (1, nb0, nb1)):
                        ci = 0
                        for g0 in range(0, nbh, KMAX):
                            G = min(KMAX, nbh - g0)
                            sel = spool.tile([P, KMAX * HD], bf16, tag="sel")
                            nc.vector.tensor_tensor(
                                out=sel[:, : G * HD],
                                in0=iota_t[:, : G * HD],
                                in1=rel_t[
                                    :, off + loc + h0 + g0 : off + loc + h0 + g0 + G
                                ].to_broadcast([P, G, HD]),
                                op=Alu.is_equal,
                            )
                            for j in range(G):
                                c = loc + h0 + g0 + j
                                nc.tensor.matmul(
                                    out=ps[half * HD : (half + 1) * HD, :],
                                    lhsT=sel[:, j * HD : (j + 1) * HD],
                                    rhs=rt[:, c * D : (c + 1) * D],
                                    start=(ci == 0), stop=(ci == nbh - 1),
                                )
                                ci += 1
                    vcol = wpool.tile([P, 1], f32, tag="vcol")
                    nc.vector.tensor_tensor(
                        out=vcol[:], in0=ex_t[:, b : b + 1], in1=rs2c[:], op=Alu.mult
                    )
                    ho = (b - HSEC[hs]) * D
                    nc.scalar.activation(
                        out=hsec_t[:, ho : ho + D], in_=ps[:], func=Act.Copy,
                        scale=vcol[:],
                    )
                    if b == HSEC[hs + 1] - 1:
                        nc.sync.dma_start(
                            out=hno[:, HSEC[hs] * D : HSEC[hs + 1] * D], in_=hsec_t[:]
                        )
                        hs += 1
                    loc += nb0 + nb1
                off += gtot
    return nc


# ---------------------------------------------------------------- host glue
def _route(key, ngroups, nblocks, payload_idx, slot):
    """Sort incidences by (core, block-half) key; pad each (block, half) to
    core-uniform chunk counts. Returns (per-core payload-index array [TOT*P],
    per-core rel array [TOT*P] (-1 = pad), cb [nblocks] chunks per group)."""
    cnt = np.bincount(key, minlength=ngroups)
    cb = np.maximum(1, -(-cnt.reshape(NC, nblocks) // P)).max(axis=0)  # ceil
    chunkbase = np.zeros(nblocks, np.int64)
    np.cumsum(cb[:-1], out=chunkbase[1:])
    TOT = int(cb.sum())
    order = np.argsort(key, kind="stable")
    ks = key[order]
    gstart = np.zeros(ngroups, np.int64)
    np.cumsum(cnt[:-1], out=gstart[1:])
    rank = np.arange(M, dtype=np.int64) - gstart[ks]
    b_s = ks % nblocks
    c_s = ks // nblocks
    pos = chunkbase[b_s] * P + rank
    pidx = np.zeros((NC, TOT * P), np.int64)
    relv = np.full((NC, TOT * P), -1.0, np.float32)
    pidx[c_s, pos] = payload_idx[order]
    relv[c_s, pos] = slot[order]
    return pidx, relv, cb


def _pack_rows(table, pidx, relv, ew):
    """Gather rows per incidence slot and lay out partition-major:
    out[p, ci*ew:(ci+1)*ew] = table[pidx[ci*P+p]]."""
    TOTP = pidx.shape[0]
    r = table[pidx]  # [TOT*P, ew]
    r = np.ascontiguousarray(
        r.reshape(TOTP // P, P, ew).transpose(1, 0, 2).reshape(P, TOTP // P * ew)
    )
    rl = np.ascontiguousarray(relv.reshape(TOTP // P, P).T.astype(BF))
    return r, rl


def kernel(x, W, b, attn_node, attn_edge, node_idx, he_idx, num_hyperedges):
    x = np.asarray(x, np.float32)
    W = np.asarray(W, np.float32)
    b = np.asarray(b, np.float32).reshape(-1)
    attn_node = np.asarray(attn_node, np.float32).reshape(-1)
    attn_edge = np.asarray(attn_edge, np.float32).reshape(-1)
    node_idx = np.asarray(node_idx).astype(np.int64)
    he_idx = np.asarray(he_idx).astype(np.int64)
    assert x.shape == (N, D) and node_idx.shape == (M,) and int(num_hyperedges) == H
    LAST_EXEC_TIMES.clear()

    iota_np = np.ascontiguousarray(
        np.tile(np.arange(P // 2, dtype=np.float32), (P, KMAX)).astype(BF)
    )
    ones_row = np.ones((1, P), BF)
    ones_col = np.ones((P, 1), BF)

    # ---------- launch A ----------
    nc_a = _build_launch_a()
    xT = np.ascontiguousarray(x.T)  # [128, N]
    cnt = np.bincount(node_idx, minlength=N).astype(np.float32)
    ins_a = []
    for c in range(NC):
        xts = np.zeros((P, NSHP), np.float32)
        xts[:, :NSH] = xT[:, c * NSH : (c + 1) * NSH]
        cnt_p = np.zeros(NSHP, np.float32)
        cnt_p[:NSH] = cnt[c * NSH : (c + 1) * NSH]
        ins_a.append(
            {
                "xT": xts,
                "W": W.astype(BF),
                "WT": np.ascontiguousarray(W.T).astype(BF),
                "b_col": b.reshape(D, 1).astype(BF),
                "an_col": attn_node.reshape(D, 1).astype(BF),
                "ones_row": ones_row,
                "ones_col": ones_col,
                "cnt_w": np.ascontiguousarray(cnt_p.reshape(NBA, P).T),
            }
        )
    res_a = _run(nc_a, ins_a, "A")
    g_full = np.concatenate(
        [
            np.asarray(res_a[c]["g_sh"])
            .reshape(P, NBA, EW)
            .transpose(1, 0, 2)
            .reshape(NSHP, EW)[:NSH]
            for c in range(NC)
        ],
        axis=0,
    )  # [N, EW] bf16
    s1p_np = np.concatenate(
        [np.asarray(res_a[c]["s1_part"]) for c in range(NC)], axis=1
    )  # [1, 8] f32

    # ---------- stage-1 routing (host halo exchange) ----------
    c1 = he_idx // HSH
    b1 = (he_idx % HSH) // P
    s1v = (he_idx % HSH) % P
    hf1 = s1v // (P // 2)
    slot1 = (s1v % (P // 2)).astype(np.float32)
    key1 = ((c1 * NBB + b1) * 2 + hf1).astype(np.int64)
    pidx1, relv1, cb1 = _route(key1, NC * NBB * 2, NBB * 2, node_idx, slot1)

    nc_b = _build_launch_b(cb1.tolist())
    ae_bc = np.tile(attn_edge.reshape(1, D), (P, 1)).astype(np.float32)
    b_bc = np.tile(b.reshape(1, D), (P, 1)).astype(np.float32)
    ins_b = []
    for c in range(NC):
        r, rl = _pack_rows(g_full, pidx1[c], relv1[c], EW)
        ins_b.append(
            {
                "rows": r,
                "rel": rl,
                "iota": iota_np,
                "ae_bc": ae_bc,
                "b_bc": b_bc,
                "s1p": np.ascontiguousarray(np.tile(s1p_np, (P, 1))),
                "ones_col": ones_col,
            }
        )
    res_b = _run(nc_b, ins_b, "B")
    t2_full = np.concatenate(
        [
            np.asarray(res_b[c]["t2o"])
            .reshape(P, NBB, D)
            .transpose(1, 0, 2)
            .reshape(NBB * P, D)[:HSH]
            for c in range(NC)
        ],
        axis=0,
    )  # [H, D] bf16
    s2p_np = np.concatenate(
        [np.asarray(res_b[c]["s2_part"]) for c in range(NC)], axis=1
    )  # [1, 8] f32

    # ---------- stage-2 routing ----------
    c2 = node_idx // NSH
    b2 = (node_idx % NSH) // P
    s2v = (node_idx % NSH) % P
    hf2 = s2v // (P // 2)
    slot2 = (s2v % (P // 2)).astype(np.float32)
    key2 = ((c2 * NBA + b2) * 2 + hf2).astype(np.int64)
    pidx2, relv2, cb2 = _route(key2, NC * NBA * 2, NBA * 2, he_idx, slot2)

    nc_c = _build_launch_c(cb2.tolist())
    ins_c = []
    for c in range(NC):
        r, rl = _pack_rows(t2_full, pidx2[c], relv2[c], D)
        ins_c.append(
            {
                "rows": r,
                "rel": rl,
                "iota": iota_np,
                "exsh": np.asarray(res_a[c]["exan_sh"]),
                "s2p": np.ascontiguousarray(np.tile(s2p_np, (P, 1))),
            }
        )
    res_c = _run(nc_c, ins_c, "C")
    h_n = np.concatenate(
        [
            np.asarray(res_c[c]["hno"])
            .reshape(P, NBA, D)
            .transpose(1, 0, 2)
            .reshape(NSHP, D)[:NSH]
            .astype(np.float32)
            for c in range(NC)
        ],
        axis=0,
    )
    return h_n


# revision 57
# speedup vs baseline: 1.0281x; 1.0281x over previous
"""HGATConv on 8 trn2 NeuronCores via Bass/Tile.

Math (equivalent to reference; softmax without max-shift — logits are small):
  h = x@W + b;  a_n = h@attn_node;  e = exp(a_n)
  stage1: hhat[j] = sum_{i: he_i=j} e[n_i]*h[n_i];  q[j] = sum_{i: he_i=j} e[n_i]
          S1 = sum_i e[n_i] = sum_n cnt[n]*e[n]
  a_e = (hhat @ attn_edge)/S1;  u = exp(a_e);  T2[j] = (u[j]/S1)*hhat[j]
  S2 = sum_j u[j]*q[j]
  h_n[n] = (e[n]/S2) * sum_{i: node_i=n} T2[he_i]

Three SPMD launches. The host performs the halo exchange between launches
(pure data movement: fancy-indexed row routing of per-incidence payloads,
sorted by destination block and padded to core-uniform chunk counts) so
every device-side DMA is a large streaming HWDGE transfer — no dma_gather.
Per-incidence payloads travel as bf16 (tolerance is 2e-2).

  A: node shard -> bf16 row table [e*(x@W) | e | pad] (132 cols), e table,
     S1 partial. One bf16 matmul per 128-node block computes [h | a_n] via
     extended weights [W | W@attn_node]; bias b is folded into launch B
     exactly via hhat_corrected = hhat + q*b.
  B: stream stage-1 rows; segment-sum via bf16 selection matmuls. Incidences
     are routed per (dest block, 64-dest half) so sel is [128, 64] (half the
     DVE is_equal work) and each matmul writes its half of the block's PSUM.
     Per-block epilogue computes u, the T2 row block, and the S2 partial.
  C: stream stage-2 T2 rows; same half-split selection matmuls; scale by
     e[n]/S2 -> h_n (bf16, host widens to fp32 losslessly)
"""
import os
import sys

sys.path.insert(0, os.path.dirname(os.path.abspath(__file__)))
try:
    import ntff_shim  # noqa: F401  (optional; enables trace under axon)
except Exception:
    pass

import numpy as np
import ml_dtypes
import concourse.bacc as bacc
import concourse.mybir as mybir
import concourse.tile as tile
from concourse.bass_utils import run_bass_kernel_spmd

f32 = mybir.dt.float32
bf16 = mybir.dt.bfloat16
BF = ml_dtypes.bfloat16
P = 128
NC = 8
N, H, M, D = 100000, 20000, 600000, 128
NSH, HSH = N // NC, H // NC          # 12500, 2500
NBA = (NSH + P - 1) // P             # 98 node blocks per core
NSHP = NBA * P                       # 12544
NBB = (HSH + P - 1) // P             # 20 hyperedge blocks per core
EW = 132                             # stage-1 row floats: [e*h(128) | e | 3 pad]
KMAX = 8                             # sel-matrix chunks per DVE op
CGRP = 8                             # stage-2 blocks per rows DMA

LAST_EXEC_TIMES = []
_TRACE = bool(os.environ.get("HGAT_TRACE"))

Alu = mybir.AluOpType
Act = mybir.ActivationFunctionType


def _run(nc, ins, tag):
    nc.finalize()
    res = run_bass_kernel_spmd(nc, ins, list(range(NC)), trace=_TRACE)
    if _TRACE:
        LAST_EXEC_TIMES.append((tag, res.exec_time_ns, res.mean_exec_time_ns))
    return res.results


# ---------------------------------------------------------------- launch A
def _build_launch_a():
    nc = bacc.Bacc("TRN2")
    xT = nc.declare_dram_parameter("xT", [P, NSHP], f32, isOutput=False)
    Wp = nc.declare_dram_parameter("W", [P, D], bf16, isOutput=False)
    WT = nc.declare_dram_parameter("WT", [P, D], bf16, isOutput=False)
    b_col = nc.declare_dram_parameter("b_col", [P, 1], bf16, isOutput=False)
    an_col = nc.declare_dram_parameter("an_col", [P, 1], bf16, isOutput=False)
    ones_row = nc.declare_dram_parameter("ones_row", [1, P], bf16, isOutput=False)
    ones_col = nc.declare_dram_parameter("ones_col", [P, 1], bf16, isOutput=False)
    cnt_w = nc.declare_dram_parameter("cnt_w", [P, NBA], f32, isOutput=False)
    g_sh = nc.declare_dram_parameter("g_sh", [P, NBA * EW], bf16, isOutput=True)
    exan_sh = nc.declare_dram_parameter("exan_sh", [P, NBA], f32, isOutput=True)
    s1_part = nc.declare_dram_parameter("s1_part", [1, 1], f32, isOutput=True)

    XSEC = [0, 25, 50, 75, NBA]  # x section boundaries (blocks)
    with tile.TileContext(nc) as tc:
        with (
            tc.tile_pool(name="sbuf", bufs=1) as pool,
            tc.tile_pool(name="work", bufs=2) as wpool,
            tc.tile_pool(name="psh", bufs=6, space="PSUM") as pph,
            tc.tile_pool(name="pscl", bufs=1, space="PSUM") as pscl,
        ):
            wbf = pool.tile([P, D], bf16)
            nc.sync.dma_start(out=wbf[:], in_=Wp[:])
            wtt = pool.tile([P, D], bf16)
            nc.sync.dma_start(out=wtt[:], in_=WT[:])
            bcol_t = pool.tile([P, 1], bf16)
            nc.sync.dma_start(out=bcol_t[:], in_=b_col[:])
            ancol_t = pool.tile([P, 1], bf16)
            nc.sync.dma_start(out=ancol_t[:], in_=an_col[:])
            onr_bf = pool.tile([1, P], bf16)
            nc.sync.dma_start(out=onr_bf[:], in_=ones_row[:])
            onc_bf = pool.tile([P, 1], bf16)
            nc.sync.dma_start(out=onc_bf[:], in_=ones_col[:])
            cnt_t = pool.tile([P, NBA], f32)
            nc.sync.dma_start(out=cnt_t[:], in_=cnt_w[:])
            xsec = []
            for s in range(4):
                c0, c1 = XSEC[s] * P, XSEC[s + 1] * P
                xs_t = pool.tile([P, c1 - c0], f32, name=f"xs{s}")
                nc.sync.dma_start(out=xs_t[:], in_=xT[:, c0:c1])
                xb_t = pool.tile([P, c1 - c0], bf16, name=f"xb{s}")
                nc.vector.tensor_copy(out=xb_t[:], in_=xs_t[:])
                xsec.append(xb_t)

            # wa = W @ attn_node (via W^T as lhsT); ab = b @ attn_node
            # wext = [W | wa] so h and a_n come from ONE matmul per block
            pw = pscl.tile([P, 1], f32, tag="scl", space="PSUM")
            nc.tensor.matmul(out=pw[:], lhsT=wtt[:], rhs=ancol_t[:], start=True, stop=True)
            wext = pool.tile([P, D + 1], bf16)
            nc.vector.tensor_copy(out=wext[:, 0:D], in_=wbf[:])
            nc.vector.tensor_copy(out=wext[:, D : D + 1], in_=pw[:])
            pab = pscl.tile([1, 1], f32, tag="scl", space="PSUM")
            nc.tensor.matmul(out=pab[:], lhsT=bcol_t[:], rhs=ancol_t[:], start=True, stop=True)
            ab_sb = pool.tile([1, 1], bf16)
            nc.vector.tensor_copy(out=ab_sb[:], in_=pab[:])
            pabc = pscl.tile([P, 1], f32, tag="scl", space="PSUM")
            nc.tensor.matmul(out=pabc[:], lhsT=onr_bf[:], rhs=ab_sb[:], start=True, stop=True)
            ab_col = pool.tile([P, 1], f32)
            nc.vector.tensor_copy(out=ab_col[:], in_=pabc[:])

            gbig0 = pool.tile([P, 49 * EW], bf16)
            gbig1 = pool.tile([P, (NBA - 49) * EW], bf16)
            nc.gpsimd.memset(gbig0[:], 0)
            nc.gpsimd.memset(gbig1[:], 0)
            exan = pool.tile([P, NBA], f32)

            for t in range(NBA):
                s = min(3, t // 25)
                xs = xsec[s][:, (t - XSEC[s]) * P : (t - XSEC[s] + 1) * P]
                gb = gbig0 if t < 49 else gbig1
                go = (t if t < 49 else t - 49) * EW
                # one matmul per block: [h | a_n] = x @ [W | wa]
                ps_h = pph.tile([P, D + 1], f32, tag="ph", space="PSUM")
                nc.tensor.matmul(out=ps_h[:], lhsT=xs, rhs=wext[:], start=True, stop=True)
                ecol = exan[:, t : t + 1]
                nc.scalar.activation(
                    out=ecol, in_=ps_h[:, D : D + 1], func=Act.Exp, bias=ab_col[:]
                )
                if t % 2 == 0:
                    # balance the e*h scale between DVE and scalar engines
                    nc.vector.tensor_scalar(
                        out=gb[:, go : go + D], in0=ps_h[:, 0:D], scalar1=ecol,
                        scalar2=None, op0=Alu.mult,
                    )
                else:
                    nc.scalar.activation(
                        out=gb[:, go : go + D], in_=ps_h[:, 0:D], func=Act.Copy,
                        scale=ecol,
                    )
                nc.vector.tensor_copy(out=gb[:, go + D : go + D + 1], in_=ecol)
                if t == 48:
                    nc.sync.dma_start(out=g_sh[:, : 49 * EW], in_=gbig0[:])
            nc.sync.dma_start(out=g_sh[:, 49 * EW :], in_=gbig1[:])
            nc.sync.dma_start(out=exan_sh[:], in_=exan[:])

            # S1 partial = sum(cnt * e) over this core's shard
            scr = wpool.tile([P, NBA], f32, tag="scr")
            s1col = pool.tile([P, 1], f32)
            nc.vector.tensor_tensor(
                out=scr[:], in0=exan[:], in1=cnt_t[:], op=Alu.mult
            )
            nc.vector.tensor_reduce(
                out=s1col[:], in_=scr[:], axis=mybir.AxisListType.X, op=Alu.add
            )
            s1bf = pool.tile([P, 1], bf16)
            nc.vector.tensor_copy(out=s1bf[:], in_=s1col[:])
            ps1 = pscl.tile([1, 1], f32, tag="scl", space="PSUM")
            nc.tensor.matmul(out=ps1[:], lhsT=s1bf[:], rhs=onc_bf[:], start=True, stop=True)
            s1sb = pool.tile([1, 1], f32)
            nc.vector.tensor_copy(out=s1sb[:], in_=ps1[:])
            nc.sync.dma_start(out=s1_part[:], in_=s1sb[:])
    return nc


# ---------------------------------------------------------------- launch B
def _build_launch_b(cb1):
    """cb1: chunks per (block, half) — length 2*NBB, order (b0,h0),(b0,h1),..."""
    TOT1 = int(sum(cb1))
    CBMAX = max(int(cb1[2 * b] + cb1[2 * b + 1]) for b in range(NBB))
    HD = P // 2
    nc = bacc.Bacc("TRN2")
    rows = nc.declare_dram_parameter("rows", [P, TOT1 * EW], bf16, isOutput=False)
    rel = nc.declare_dram_parameter("rel", [P, TOT1], bf16, isOutput=False)
    iota = nc.declare_dram_parameter("iota", [P, KMAX * HD], bf16, isOutput=False)
    ae_bc = nc.declare_dram_parameter("ae_bc", [P, D], f32, isOutput=False)
    b_bc = nc.declare_dram_parameter("b_bc", [P, D], f32, isOutput=False)
    s1p = nc.declare_dram_parameter("s1p", [P, NC], f32, isOutput=False)
    ones_col = nc.declare_dram_parameter("ones_col", [P, 1], bf16, isOutput=False)
    t2o = nc.declare_dram_parameter("t2o", [P, NBB * D], bf16, isOutput=True)
    s2_part = nc.declare_dram_parameter("s2_part", [1, 1], f32, isOutput=True)

    with tile.TileContext(nc) as tc:
        with (
            tc.tile_pool(name="sbuf", bufs=1) as pool,
            tc.tile_pool(name="rows", bufs=4) as rpool,
            tc.tile_pool(name="sel", bufs=4) as spool,
            tc.tile_pool(name="work", bufs=2) as wpool,
            tc.tile_pool(name="psum", bufs=2, space="PSUM") as pp,
            tc.tile_pool(name="pscl", bufs=1, space="PSUM") as pscl,
        ):
            rel_t = pool.tile([P, TOT1], bf16)
            nc.sync.dma_start(out=rel_t[:], in_=rel[:])
            iota_t = pool.tile([P, KMAX * HD], bf16)
            nc.sync.dma_start(out=iota_t[:], in_=iota[:])
            ae_t = pool.tile([P, D], f32)
            nc.sync.dma_start(out=ae_t[:], in_=ae_bc[:])
            bb_t = pool.tile([P, D], f32)
            nc.sync.dma_start(out=bb_t[:], in_=b_bc[:])
            s1p_t = pool.tile([P, NC], f32)
            nc.sync.dma_start(out=s1p_t[:], in_=s1p[:])
            onc_bf = pool.tile([P, 1], bf16)
            nc.sync.dma_start(out=onc_bf[:], in_=ones_col[:])

            s1tot = pool.tile([P, 1], f32)
            nc.vector.tensor_reduce(
                out=s1tot[:], in_=s1p_t[:], axis=mybir.AxisListType.X, op=Alu.add
            )
            rs1c = pool.tile([P, 1], f32)
            nc.vector.reciprocal(out=rs1c[:], in_=s1tot[:])

            t2big = pool.tile([P, NBB * D], bf16)
            s2acc = pool.tile([P, 1], f32)
            nc.vector.memset(s2acc[:], 0)

            off = 0
            for b in range(NBB):
                nb0, nb1 = int(cb1[2 * b]), int(cb1[2 * b + 1])
                nb = nb0 + nb1
                rt = rpool.tile([P, CBMAX * EW], bf16, tag="rows")
                nc.sync.dma_start(
                    out=rt[:, : nb * EW], in_=rows[:, off * EW : (off + nb) * EW]
                )
                ps = pp.tile([P, D + 1], f32, tag="ps", space="PSUM")
                for half, h0, nbh in ((0, 0, nb0), (1, nb0, nb1)):
                    ci = 0
                    for g0 in range(0, nbh, KMAX):
                        G = min(KMAX, nbh - g0)
                        sel = spool.tile([P, KMAX * HD], bf16, tag="sel")
                        nc.vector.tensor_tensor(
                            out=sel[:, : G * HD],
                            in0=iota_t[:, : G * HD],
                            in1=rel_t[
                                :, off + h0 + g0 : off + h0 + g0 + G
                            ].to_broadcast([P, G, HD]),
                            op=Alu.is_equal,
                        )
                        for j in range(G):
                            c = h0 + g0 + j
                            nc.tensor.matmul(
                                out=ps[half * HD : (half + 1) * HD, :],
                                lhsT=sel[:, j * HD : (j + 1) * HD],
                                rhs=rt[:, c * EW : c * EW + D + 1],
                                start=(ci == 0), stop=(ci == nbh - 1),
                            )
                            ci += 1
                # epilogue: hh = hhat + q*b, then a_e, u, T2 block, S2 partial
                hh = wpool.tile([P, D], f32, tag="hh")
                nc.vector.scalar_tensor_tensor(
                    out=hh[:], in0=bb_t[:], scalar=ps[:, D : D + 1], in1=ps[:, 0:D],
                    op0=Alu.mult, op1=Alu.add,
                )
                scr = wpool.tile([P, D], f32, tag="scr")
                araw = wpool.tile([P, 1], f32, tag="araw")
                nc.vector.tensor_tensor(
                    out=scr[:], in0=hh[:], in1=ae_t[:], op=Alu.mult
                )
                nc.vector.tensor_reduce(
                    out=araw[:], in_=scr[:], axis=mybir.AxisListType.X, op=Alu.add
                )
                ucol = wpool.tile([P, 1], f32, tag="ucol")
                nc.scalar.activation(out=ucol[:], in_=araw[:], func=Act.Exp, scale=rs1c[:])
                wcol = wpool.tile([P, 1], f32, tag="wcol")
                nc.vector.tensor_tensor(
                    out=wcol[:], in0=ucol[:], in1=rs1c[:], op=Alu.mult
                )
                nc.scalar.activation(
                    out=t2big[:, b * D : (b + 1) * D], in_=hh[:],
                    func=Act.Copy, scale=wcol[:],
                )
                nc.vector.scalar_tensor_tensor(
                    out=s2acc[:], in0=ucol[:], scalar=ps[:, D : D + 1], in1=s2acc[:],
                    op0=Alu.mult, op1=Alu.add,
                )
                off += nb

            nc.sync.dma_start(out=t2o[:], in_=t2big[:])
            s2bf = pool.tile([P, 1], bf16)
            nc.vector.tensor_copy(out=s2bf[:], in_=s2acc[:])
            ps2 = pscl.tile([1, 1], f32, tag="ps2", space="PSUM")
            nc.tensor.matmul(out=ps2[:], lhsT=s2bf[:], rhs=onc_bf[:], start=True, stop=True)
            s2sb = pool.tile([1, 1], f32)
            nc.vector.tensor_copy(out=s2sb[:], in_=ps2[:])
            nc.sync.dma_start(out=s2_part[:], in_=s2sb[:])
    return nc


# ---------------------------------------------------------------- launch C
def _build_launch_c(cb2):
    """cb2: chunks per (block, half) — length 2*NBA."""
    TOT2 = int(sum(cb2))
    HD = P // 2
    nblk = [int(cb2[2 * b] + cb2[2 * b + 1]) for b in range(NBA)]
    # rows DMA groups of CGRP blocks
    groups = []
    for g0 in range(0, NBA, CGRP):
        blks = list(range(g0, min(NBA, g0 + CGRP)))
        groups.append(blks)
    GMAX = max(sum(nblk[b] for b in blks) for blks in groups)
    HSEC = [0, 25, 50, 75, NBA]  # h_n output sections

    nc = bacc.Bacc("TRN2")
    rows = nc.declare_dram_parameter("rows", [P, TOT2 * D], bf16, isOutput=False)
    rel = nc.declare_dram_parameter("rel", [P, TOT2], bf16, isOutput=False)
    iota = nc.declare_dram_parameter("iota", [P, KMAX * HD], bf16, isOutput=False)
    exsh = nc.declare_dram_parameter("exsh", [P, NBA], f32, isOutput=False)
    s2p = nc.declare_dram_parameter("s2p", [P, NC], f32, isOutput=False)
    hno = nc.declare_dram_parameter("hno", [P, NBA * D], bf16, isOutput=True)

    with tile.TileContext(nc) as tc:
        with (
            tc.tile_pool(name="sbuf", bufs=1) as pool,
            tc.tile_pool(name="rows", bufs=4) as rpool,
            tc.tile_pool(name="sel", bufs=4) as spool,
            tc.tile_pool(name="work", bufs=2) as wpool,
            tc.tile_pool(name="hsec", bufs=2) as hpool,
            tc.tile_pool(name="psum", bufs=2, space="PSUM") as pp,
        ):
            rel_t = pool.tile([P, TOT2], bf16)
            nc.sync.dma_start(out=rel_t[:], in_=rel[:])
            iota_t = pool.tile([P, KMAX * HD], bf16)
            nc.sync.dma_start(out=iota_t[:], in_=iota[:])
            ex_t = pool.tile([P, NBA], f32)
            nc.sync.dma_start(out=ex_t[:], in_=exsh[:])
            s2p_t = pool.tile([P, NC], f32)
            nc.sync.dma_start(out=s2p_t[:], in_=s2p[:])

            s2tot = pool.tile([P, 1], f32)
            nc.vector.tensor_reduce(
                out=s2tot[:], in_=s2p_t[:], axis=mybir.AxisListType.X, op=Alu.add
            )
            rs2c = pool.tile([P, 1], f32)
            nc.vector.reciprocal(out=rs2c[:], in_=s2tot[:])

            hsec_t = None
            hs = 0
            off = 0
            for blks in groups:
                gtot = sum(nblk[b] for b in blks)
                rt = rpool.tile([P, GMAX * D], bf16, tag="rows")
                nc.sync.dma_start(
                    out=rt[:, : gtot * D], in_=rows[:, off * D : (off + gtot) * D]
                )
                loc = 0
                for b in blks:
                    nb0, nb1 = int(cb2[2 * b]), int(cb2[2 * b + 1])
                    if b == HSEC[hs]:
                        hsec_t = hpool.tile(
                            [P, (HSEC[hs + 1] - HSEC[hs]) * D], bf16, tag="hsec"
                        )
                    ps = pp.tile([P, D], f32, tag="ps", space="PSUM")
                    for half, h0, nbh in ((0, 0, nb0), (1, nb0, nb1)):
                        ci = 0
                        for g0 in range(0, nbh, KMAX):
                            G = min(KMAX, nbh - g0)
                            sel = spool.tile([P, KMAX * HD], bf16, tag="sel")
                            nc.vector.tensor_tensor(
                                out=sel[:, : G * HD],
                                in0=iota_t[:, : G * HD],
                                in1=rel_t[
                                    :, off + loc + h0 + g0 : off + loc + h0 + g0 + G
                                ].to_broadcast([P, G, HD]),
                                op=Alu.is_equal,
                            )
                            for j in range(G):
                                c = loc + h0 + g0 + j
                                nc.tensor.matmul(
                                    out=ps[half * HD : (half + 1) * HD, :],
                                    lhsT=sel[:, j * HD : (j + 1) * HD],
                                    rhs=rt[:, c * D : (c + 1) * D],
                                    start=(ci == 0), stop=(ci == nbh - 1),
                                )
                                ci += 1
                    vcol = wpool.tile([P, 1], f32, tag="vcol")
                    nc.vector.tensor_tensor(
                        out=vcol[:], in0=ex_t[:, b : b + 1], in1=rs2c[:], op=Alu.mult
                    )
                    ho = (b - HSEC[hs]) * D
                    nc.scalar.activation(
                        out=hsec_t[:, ho : ho + D], in_=ps[:], func=Act.Copy,
                        scale=vcol[:],
                    )
                    if b == HSEC[hs + 1] - 1:
                        nc.sync.dma_start(
                            out=hno[:, HSEC[hs] * D : HSEC[hs + 1] * D], in_=hsec_t[:]
                        )
                        hs += 1
                    loc += nb0 + nb1
                off += gtot
    return nc


# ---------------------------------------------------------------- host glue
def _route(key, ngroups, nblocks, payload_idx, slot):
    """Sort incidences by (core, block-half) key; pad each (block, half) to
    core-uniform chunk counts. Returns (per-core payload-index array [TOT*P],
    per-core rel array [TOT*P] (-1 = pad), cb [nblocks] chunks per group)."""
    cnt = np.bincount(key, minlength=ngroups)
    cb = np.maximum(1, -(-cnt.reshape(NC, nblocks) // P)).max(axis=0)  # ceil
    chunkbase = np.zeros(nblocks, np.int64)
    np.cumsum(cb[:-1], out=chunkbase[1:])
    TOT = int(cb.sum())
    order = np.argsort(key, kind="stable")
    ks = key[order]
    gstart = np.zeros(ngroups, np.int64)
    np.cumsum(cnt[:-1], out=gstart[1:])
    rank = np.arange(M, dtype=np.int64) - gstart[ks]
    b_s = ks % nblocks
    c_s = ks // nblocks
    pos = chunkbase[b_s] * P + rank
    pidx = np.zeros((NC, TOT * P), np.int64)
    relv = np.full((NC, TOT * P), -1.0, np.float32)
    pidx[c_s, pos] = payload_idx[order]
    relv[c_s, pos] = slot[order]
    return pidx, relv, cb


def _pack_rows(table, pidx, relv, ew):
    """Gather rows per incidence slot and lay out partition-major:
    out[p, ci*ew:(ci+1)*ew] = table[pidx[ci*P+p]]."""
    TOTP = pidx.shape[0]
    r = table[pidx]  # [TOT*P, ew]
    r = np.ascontiguousarray(
        r.reshape(TOTP // P, P, ew).transpose(1, 0, 2).reshape(P, TOTP // P * ew)
    )
    rl = np.ascontiguousarray(relv.reshape(TOTP // P, P).T.astype(BF))
    return r, rl


def kernel(x, W, b, attn_node, attn_edge, node_idx, he_idx, num_hyperedges):
    x = np.asarray(x, np.float32)
    W = np.asarray(W, np.float32)
    b = np.asarray(b, np.float32).reshape(-1)
    attn_node = np.asarray(attn_node, np.float32).reshape(-1)
    attn_edge = np.asarray(attn_edge, np.float32).reshape(-1)
    node_idx = np.asarray(node_idx).astype(np.int64)
    he_idx = np.asarray(he_idx).astype(np.int64)
    assert x.shape == (N, D) and node_idx.shape == (M,) and int(num_hyperedges) == H
    LAST_EXEC_TIMES.clear()

    iota_np = np.ascontiguousarray(
        np.tile(np.arange(P // 2, dtype=np.float32), (P, KMAX)).astype(BF)
    )
    ones_row = np.ones((1, P), BF)
    ones_col = np.ones((P, 1), BF)

    # ---------- launch A ----------
    nc_a = _build_launch_a()
    xT = np.ascontiguousarray(x.T)  # [128, N]
    cnt = np.bincount(node_idx, minlength=N).astype(np.float32)
    ins_a = []
    for c in range(NC):
        xts = np.zeros((P, NSHP), np.float32)
        xts[:, :NSH] = xT[:, c * NSH : (c + 1) * NSH]
        cnt_p = np.zeros(NSHP, np.float32)
        cnt_p[:NSH] = cnt[c * NSH : (c + 1) * NSH]
        ins_a.append(
            {
                "xT": xts,
                "W": W.astype(BF),
                "WT": np.ascontiguousarray(W.T).astype(BF),
                "b_col": b.reshape(D, 1).astype(BF),
                "an_col": attn_node.reshape(D, 1).astype(BF),
                "ones_row": ones_row,
                "ones_col": ones_col,
                "cnt_w": np.ascontiguousarray(cnt_p.reshape(NBA, P).T),
            }
        )
    res_a = _run(nc_a, ins_a, "A")
    g_full = np.concatenate(
        [
            np.asarray(res_a[c]["g_sh"])
            .reshape(P, NBA, EW)
            .transpose(1, 0, 2)
            .reshape(NSHP, EW)[:NSH]
            for c in range(NC)
        ],
        axis=0,
    )  # [N, EW] bf16
    s1p_np = np.concatenate(
        [np.asarray(res_a[c]["s1_part"]) for c in range(NC)], axis=1
    )  # [1, 8] f32

    # ---------- stage-1 routing (host halo exchange) ----------
    c1 = he_idx // HSH
    b1 = (he_idx % HSH) // P
    s1v = (he_idx % HSH) % P
    hf1 = s1v // (P // 2)
    slot1 = (s1v % (P // 2)).astype(np.float32)
    key1 = ((c1 * NBB + b1) * 2 + hf1).astype(np.int64)
    pidx1, relv1, cb1 = _route(key1, NC * NBB * 2, NBB * 2, node_idx, slot1)

    nc_b = _build_launch_b(cb1.tolist())
    ae_bc = np.tile(attn_edge.reshape(1, D), (P, 1)).astype(np.float32)
    b_bc = np.tile(b.reshape(1, D), (P, 1)).astype(np.float32)
    ins_b = []
    for c in range(NC):
        r, rl = _pack_rows(g_full, pidx1[c], relv1[c], EW)
        ins_b.append(
            {
                "rows": r,
                "rel": rl,
                "iota": iota_np,
                "ae_bc": ae_bc,
                "b_bc": b_bc,
                "s1p": np.ascontiguousarray(np.tile(s1p_np, (P, 1))),
                "ones_col": ones_col,
            }
        )
    res_b = _run(nc_b, ins_b, "B")
    t2_full = np.concatenate(
        [
            np.asarray(res_b[c]["t2o"])
            .reshape(P, NBB, D)
            .transpose(1, 0, 2)
            .reshape(NBB * P, D)[:HSH]
            for c in range(NC)
        ],
        axis=0,
    )  # [H, D] bf16
    s2p_np = np.concatenate(
        [np.asarray(res_b[c]["s2_part"]) for c in range(NC)], axis=1
    )  # [1, 8] f32

    # ---------- stage-2 routing ----------
    c2 = node_idx // NSH
    b2 = (node_idx % NSH) // P
    s2v = (node_idx % NSH) % P
    hf2 = s2v // (P // 2)
    slot2 = (s2v % (P // 2)).astype(np.float32)
    key2 = ((c2 * NBA + b2) * 2 + hf2).astype(np.int64)
    pidx2, relv2, cb2 = _route(key2, NC * NBA * 2, NBA * 2, he_idx, slot2)

    nc_c = _build_launch_c(cb2.tolist())
    ins_c = []
    for c in range(NC):
        r, rl = _pack_rows(t2_full, pidx2[c], relv2[c], D)
        ins_c.append(
            {
                "rows": r,
                "rel": rl,
                "iota": iota_np,
                "exsh": np.asarray(res_a[c]["exan_sh"]),
                "s2p": np.ascontiguousarray(np.tile(s2p_np, (P, 1))),
            }
        )
    res_c = _run(nc_c, ins_c, "C")
    h_n = np.concatenate(
        [
            np.asarray(res_c[c]["hno"])
            .reshape(P, NBA, D)
            .transpose(1, 0, 2)
            .reshape(NSHP, D)[:NSH]
            .astype(np.float32)
            for c in range(NC)
        ],
        axis=0,
    )
    return h_n


# revision 58
# speedup vs baseline: 1.0344x; 1.0062x over previous
"""HGATConv on 8 trn2 NeuronCores via Bass/Tile.

Math (equivalent to reference; softmax without max-shift — logits are small):
  h = x@W + b;  a_n = h@attn_node;  e = exp(a_n)
  stage1: hhat[j] = sum_{i: he_i=j} e[n_i]*h[n_i];  q[j] = sum_{i: he_i=j} e[n_i]
          S1 = sum_i e[n_i] = sum_n cnt[n]*e[n]
  a_e = (hhat @ attn_edge)/S1;  u = exp(a_e);  T2[j] = (u[j]/S1)*hhat[j]
  S2 = sum_j u[j]*q[j]
  h_n[n] = (e[n]/S2) * sum_{i: node_i=n} T2[he_i]

Three SPMD launches. The host performs the halo exchange between launches
(pure data movement: fancy-indexed row routing of per-incidence payloads,
sorted by destination block and padded to core-uniform chunk counts) so
every device-side DMA is a large streaming HWDGE transfer — no dma_gather.
Per-incidence payloads travel as bf16 (tolerance is 2e-2).

  A: node shard -> bf16 row table [e*(x@W) | e | pad] (132 cols), e table,
     S1 partial. One bf16 matmul per 128-node block computes [h | a_n] via
     extended weights [W | W@attn_node]; bias b is folded into launch B
     exactly via hhat_corrected = hhat + q*b.
  B: stream stage-1 rows; segment-sum via bf16 selection matmuls. Incidences
     are routed per (dest block, 64-dest half) so sel is [128, 64] (half the
     DVE is_equal work) and each matmul writes its half of the block's PSUM.
     Per-block epilogue computes u, the T2 row block, and the S2 partial.
  C: stream stage-2 T2 rows; same half-split selection matmuls; scale by
     e[n]/S2 -> h_n (bf16, host widens to fp32 losslessly)
"""
import os
import sys

sys.path.insert(0, os.path.dirname(os.path.abspath(__file__)))
try:
    import ntff_shim  # noqa: F401  (optional; enables trace under axon)
except Exception:
    pass

import numpy as np
import ml_dtypes
import concourse.bacc as bacc
import concourse.mybir as mybir
import concourse.tile as tile
from concourse.bass_utils import run_bass_kernel_spmd

f32 = mybir.dt.float32
bf16 = mybir.dt.bfloat16
BF = ml_dtypes.bfloat16
P = 128
NC = 8
N, H, M, D = 100000, 20000, 600000, 128
NSH, HSH = N // NC, H // NC          # 12500, 2500
NBA = (NSH + P - 1) // P             # 98 node blocks per core
NSHP = NBA * P                       # 12544
NBB = (HSH + P - 1) // P             # 20 hyperedge blocks per core
EW = 132                             # stage-1 row floats: [e*h(128) | e | 3 pad]
KMAX = 8                             # sel-matrix chunks per DVE op
CGRP = 8                             # stage-2 blocks per rows DMA

LAST_EXEC_TIMES = []
_TRACE = bool(os.environ.get("HGAT_TRACE"))

Alu = mybir.AluOpType
Act = mybir.ActivationFunctionType


def _run(nc, ins, tag):
    nc.finalize()
    res = run_bass_kernel_spmd(nc, ins, list(range(NC)), trace=_TRACE)
    if _TRACE:
        LAST_EXEC_TIMES.append((tag, res.exec_time_ns, res.mean_exec_time_ns))
    return res.results


# ---------------------------------------------------------------- launch A
def _build_launch_a():
    nc = bacc.Bacc("TRN2")
    xT = nc.declare_dram_parameter("xT", [P, NSHP], f32, isOutput=False)
    Wp = nc.declare_dram_parameter("W", [P, D], bf16, isOutput=False)
    WT = nc.declare_dram_parameter("WT", [P, D], bf16, isOutput=False)
    b_col = nc.declare_dram_parameter("b_col", [P, 1], bf16, isOutput=False)
    an_col = nc.declare_dram_parameter("an_col", [P, 1], bf16, isOutput=False)
    ones_row = nc.declare_dram_parameter("ones_row", [1, P], bf16, isOutput=False)
    ones_col = nc.declare_dram_parameter("ones_col", [P, 1], bf16, isOutput=False)
    cnt_w = nc.declare_dram_parameter("cnt_w", [P, NBA], f32, isOutput=False)
    g_sh = nc.declare_dram_parameter("g_sh", [P, NBA * EW], bf16, isOutput=True)
    exan_sh = nc.declare_dram_parameter("exan_sh", [P, NBA], f32, isOutput=True)
    s1_part = nc.declare_dram_parameter("s1_part", [1, 1], f32, isOutput=True)

    XSEC = [0, 25, 50, 75, NBA]  # x section boundaries (blocks)
    with tile.TileContext(nc) as tc:
        with (
            tc.tile_pool(name="sbuf", bufs=1) as pool,
            tc.tile_pool(name="work", bufs=2) as wpool,
            tc.tile_pool(name="psh", bufs=6, space="PSUM") as pph,
            tc.tile_pool(name="pscl", bufs=1, space="PSUM") as pscl,
        ):
            wbf = pool.tile([P, D], bf16)
            nc.sync.dma_start(out=wbf[:], in_=Wp[:])
            wtt = pool.tile([P, D], bf16)
            nc.sync.dma_start(out=wtt[:], in_=WT[:])
            bcol_t = pool.tile([P, 1], bf16)
            nc.sync.dma_start(out=bcol_t[:], in_=b_col[:])
            ancol_t = pool.tile([P, 1], bf16)
            nc.sync.dma_start(out=ancol_t[:], in_=an_col[:])
            onr_bf = pool.tile([1, P], bf16)
            nc.sync.dma_start(out=onr_bf[:], in_=ones_row[:])
            onc_bf = pool.tile([P, 1], bf16)
            nc.sync.dma_start(out=onc_bf[:], in_=ones_col[:])
            cnt_t = pool.tile([P, NBA], f32)
            nc.sync.dma_start(out=cnt_t[:], in_=cnt_w[:])
            xsec = []
            for s in range(4):
                c0, c1 = XSEC[s] * P, XSEC[s + 1] * P
                xs_t = pool.tile([P, c1 - c0], f32, name=f"xs{s}")
                nc.sync.dma_start(out=xs_t[:], in_=xT[:, c0:c1])
                xb_t = pool.tile([P, c1 - c0], bf16, name=f"xb{s}")
                nc.vector.tensor_copy(out=xb_t[:], in_=xs_t[:])
                xsec.append(xb_t)

            # wa = W @ attn_node (via W^T as lhsT); ab = b @ attn_node
            # wext = [W | wa] so h and a_n come from ONE matmul per block
            pw = pscl.tile([P, 1], f32, tag="scl", space="PSUM")
            nc.tensor.matmul(out=pw[:], lhsT=wtt[:], rhs=ancol_t[:], start=True, stop=True)
            wext = pool.tile([P, D + 1], bf16)
            nc.vector.tensor_copy(out=wext[:, 0:D], in_=wbf[:])
            nc.vector.tensor_copy(out=wext[:, D : D + 1], in_=pw[:])
            pab = pscl.tile([1, 1], f32, tag="scl", space="PSUM")
            nc.tensor.matmul(out=pab[:], lhsT=bcol_t[:], rhs=ancol_t[:], start=True, stop=True)
            ab_sb = pool.tile([1, 1], bf16)
            nc.vector.tensor_copy(out=ab_sb[:], in_=pab[:])
            pabc = pscl.tile([P, 1], f32, tag="scl", space="PSUM")
            nc.tensor.matmul(out=pabc[:], lhsT=onr_bf[:], rhs=ab_sb[:], start=True, stop=True)
            ab_col = pool.tile([P, 1], f32)
            nc.vector.tensor_copy(out=ab_col[:], in_=pabc[:])

            gbig0 = pool.tile([P, 49 * EW], bf16)
            gbig1 = pool.tile([P, (NBA - 49) * EW], bf16)
            nc.gpsimd.memset(gbig0[:], 0)
            nc.gpsimd.memset(gbig1[:], 0)
            exan = pool.tile([P, NBA], f32)

            for t in range(NBA):
                s = min(3, t // 25)
                xs = xsec[s][:, (t - XSEC[s]) * P : (t - XSEC[s] + 1) * P]
                gb = gbig0 if t < 49 else gbig1
                go = (t if t < 49 else t - 49) * EW
                # one matmul per block: [h | a_n] = x @ [W | wa]
                ps_h = pph.tile([P, D + 1], f32, tag="ph", space="PSUM")
                nc.tensor.matmul(out=ps_h[:], lhsT=xs, rhs=wext[:], start=True, stop=True)
                ecol = exan[:, t : t + 1]
                nc.scalar.activation(
                    out=ecol, in_=ps_h[:, D : D + 1], func=Act.Exp, bias=ab_col[:]
                )
                if t % 2 == 0:
                    # balance the e*h scale between DVE and scalar engines
                    nc.vector.tensor_scalar(
                        out=gb[:, go : go + D], in0=ps_h[:, 0:D], scalar1=ecol,
                        scalar2=None, op0=Alu.mult,
                    )
                else:
                    nc.scalar.activation(
                        out=gb[:, go : go + D], in_=ps_h[:, 0:D], func=Act.Copy,
                        scale=ecol,
                    )
                nc.vector.tensor_copy(out=gb[:, go + D : go + D + 1], in_=ecol)
                if t == 48:
                    nc.sync.dma_start(out=g_sh[:, : 49 * EW], in_=gbig0[:])
            nc.sync.dma_start(out=g_sh[:, 49 * EW :], in_=gbig1[:])
            nc.sync.dma_start(out=exan_sh[:], in_=exan[:])

            # S1 partial = sum(cnt * e) over this core's shard
            scr = wpool.tile([P, NBA], f32, tag="scr")
            s1col = pool.tile([P, 1], f32)
            nc.vector.tensor_tensor(
                out=scr[:], in0=exan[:], in1=cnt_t[:], op=Alu.mult
            )
            nc.vector.tensor_reduce(
                out=s1col[:], in_=scr[:], axis=mybir.AxisListType.X, op=Alu.add
            )
            s1bf = pool.tile([P, 1], bf16)
            nc.vector.tensor_copy(out=s1bf[:], in_=s1col[:])
            ps1 = pscl.tile([1, 1], f32, tag="scl", space="PSUM")
            nc.tensor.matmul(out=ps1[:], lhsT=s1bf[:], rhs=onc_bf[:], start=True, stop=True)
            s1sb = pool.tile([1, 1], f32)
            nc.vector.tensor_copy(out=s1sb[:], in_=ps1[:])
            nc.sync.dma_start(out=s1_part[:], in_=s1sb[:])
    return nc


# ---------------------------------------------------------------- launch B
def _build_launch_b(cb1):
    """cb1: chunks per (block, half) — length 2*NBB, order (b0,h0),(b0,h1),..."""
    TOT1 = int(sum(cb1))
    CBMAX = max(int(cb1[2 * b] + cb1[2 * b + 1]) for b in range(NBB))
    HD = P // 2
    nc = bacc.Bacc("TRN2")
    rows = nc.declare_dram_parameter("rows", [P, TOT1 * EW], bf16, isOutput=False)
    rel = nc.declare_dram_parameter("rel", [P, TOT1], bf16, isOutput=False)
    iota = nc.declare_dram_parameter("iota", [P, KMAX * HD], bf16, isOutput=False)
    ae_bc = nc.declare_dram_parameter("ae_bc", [P, D], f32, isOutput=False)
    b_bc = nc.declare_dram_parameter("b_bc", [P, D], f32, isOutput=False)
    s1p = nc.declare_dram_parameter("s1p", [P, NC], f32, isOutput=False)
    ones_col = nc.declare_dram_parameter("ones_col", [P, 1], bf16, isOutput=False)
    t2o = nc.declare_dram_parameter("t2o", [P, NBB * D], bf16, isOutput=True)
    s2_part = nc.declare_dram_parameter("s2_part", [1, 1], f32, isOutput=True)

    with tile.TileContext(nc) as tc:
        with (
            tc.tile_pool(name="sbuf", bufs=1) as pool,
            tc.tile_pool(name="rows", bufs=5) as rpool,
            tc.tile_pool(name="sel", bufs=6) as spool,
            tc.tile_pool(name="work", bufs=2) as wpool,
            tc.tile_pool(name="psum", bufs=2, space="PSUM") as pp,
            tc.tile_pool(name="pscl", bufs=1, space="PSUM") as pscl,
        ):
            rel_t = pool.tile([P, TOT1], bf16)
            nc.sync.dma_start(out=rel_t[:], in_=rel[:])
            iota_t = pool.tile([P, KMAX * HD], bf16)
            nc.sync.dma_start(out=iota_t[:], in_=iota[:])
            ae_t = pool.tile([P, D], f32)
            nc.sync.dma_start(out=ae_t[:], in_=ae_bc[:])
            bb_t = pool.tile([P, D], f32)
            nc.sync.dma_start(out=bb_t[:], in_=b_bc[:])
            s1p_t = pool.tile([P, NC], f32)
            nc.sync.dma_start(out=s1p_t[:], in_=s1p[:])
            onc_bf = pool.tile([P, 1], bf16)
            nc.sync.dma_start(out=onc_bf[:], in_=ones_col[:])

            s1tot = pool.tile([P, 1], f32)
            nc.vector.tensor_reduce(
                out=s1tot[:], in_=s1p_t[:], axis=mybir.AxisListType.X, op=Alu.add
            )
            rs1c = pool.tile([P, 1], f32)
            nc.vector.reciprocal(out=rs1c[:], in_=s1tot[:])

            t2big = pool.tile([P, NBB * D], bf16)
            s2acc = pool.tile([P, 1], f32)
            nc.vector.memset(s2acc[:], 0)

            off = 0
            for b in range(NBB):
                nb0, nb1 = int(cb1[2 * b]), int(cb1[2 * b + 1])
                nb = nb0 + nb1
                rt = rpool.tile([P, CBMAX * EW], bf16, tag="rows")
                nc.sync.dma_start(
                    out=rt[:, : nb * EW], in_=rows[:, off * EW : (off + nb) * EW]
                )
                ps = pp.tile([P, D + 1], f32, tag="ps", space="PSUM")
                for half, h0, nbh in ((0, 0, nb0), (1, nb0, nb1)):
                    ci = 0
                    for g0 in range(0, nbh, KMAX):
                        G = min(KMAX, nbh - g0)
                        sel = spool.tile([P, KMAX * HD], bf16, tag="sel")
                        nc.vector.tensor_tensor(
                            out=sel[:, : G * HD],
                            in0=iota_t[:, : G * HD],
                            in1=rel_t[
                                :, off + h0 + g0 : off + h0 + g0 + G
                            ].to_broadcast([P, G, HD]),
                            op=Alu.is_equal,
                        )
                        for j in range(G):
                            c = h0 + g0 + j
                            nc.tensor.matmul(
                                out=ps[half * HD : (half + 1) * HD, :],
                                lhsT=sel[:, j * HD : (j + 1) * HD],
                                rhs=rt[:, c * EW : c * EW + D + 1],
                                start=(ci == 0), stop=(ci == nbh - 1),
                            )
                            ci += 1
                # epilogue: hh = hhat + q*b, then a_e, u, T2 block, S2 partial
                hh = wpool.tile([P, D], f32, tag="hh")
                nc.vector.scalar_tensor_tensor(
                    out=hh[:], in0=bb_t[:], scalar=ps[:, D : D + 1], in1=ps[:, 0:D],
                    op0=Alu.mult, op1=Alu.add,
                )
                scr = wpool.tile([P, D], f32, tag="scr")
                araw = wpool.tile([P, 1], f32, tag="araw")
                nc.vector.tensor_tensor(
                    out=scr[:], in0=hh[:], in1=ae_t[:], op=Alu.mult
                )
                nc.vector.tensor_reduce(
                    out=araw[:], in_=scr[:], axis=mybir.AxisListType.X, op=Alu.add
                )
                ucol = wpool.tile([P, 1], f32, tag="ucol")
                nc.scalar.activation(out=ucol[:], in_=araw[:], func=Act.Exp, scale=rs1c[:])
                wcol = wpool.tile([P, 1], f32, tag="wcol")
                nc.vector.tensor_tensor(
                    out=wcol[:], in0=ucol[:], in1=rs1c[:], op=Alu.mult
                )
                nc.scalar.activation(
                    out=t2big[:, b * D : (b + 1) * D], in_=hh[:],
                    func=Act.Copy, scale=wcol[:],
                )
                nc.vector.scalar_tensor_tensor(
                    out=s2acc[:], in0=ucol[:], scalar=ps[:, D : D + 1], in1=s2acc[:],
                    op0=Alu.mult, op1=Alu.add,
                )
                off += nb

            nc.sync.dma_start(out=t2o[:], in_=t2big[:])
            s2bf = pool.tile([P, 1], bf16)
            nc.vector.tensor_copy(out=s2bf[:], in_=s2acc[:])
            ps2 = pscl.tile([1, 1], f32, tag="ps2", space="PSUM")
            nc.tensor.matmul(out=ps2[:], lhsT=s2bf[:], rhs=onc_bf[:], start=True, stop=True)
            s2sb = pool.tile([1, 1], f32)
            nc.vector.tensor_copy(out=s2sb[:], in_=ps2[:])
            nc.sync.dma_start(out=s2_part[:], in_=s2sb[:])
    return nc


# ---------------------------------------------------------------- launch C
def _build_launch_c(cb2):
    """cb2: chunks per (block, half) — length 2*NBA."""
    TOT2 = int(sum(cb2))
    HD = P // 2
    nblk = [int(cb2[2 * b] + cb2[2 * b + 1]) for b in range(NBA)]
    # rows DMA groups of CGRP blocks
    groups = []
    for g0 in range(0, NBA, CGRP):
        blks = list(range(g0, min(NBA, g0 + CGRP)))
        groups.append(blks)
    GMAX = max(sum(nblk[b] for b in blks) for blks in groups)
    HSEC = [0, 25, 50, 75, NBA]  # h_n output sections

    nc = bacc.Bacc("TRN2")
    rows = nc.declare_dram_parameter("rows", [P, TOT2 * D], bf16, isOutput=False)
    rel = nc.declare_dram_parameter("rel", [P, TOT2], bf16, isOutput=False)
    iota = nc.declare_dram_parameter("iota", [P, KMAX * HD], bf16, isOutput=False)
    exsh = nc.declare_dram_parameter("exsh", [P, NBA], f32, isOutput=False)
    s2p = nc.declare_dram_parameter("s2p", [P, NC], f32, isOutput=False)
    hno = nc.declare_dram_parameter("hno", [P, NBA * D], bf16, isOutput=True)

    with tile.TileContext(nc) as tc:
        with (
            tc.tile_pool(name="sbuf", bufs=1) as pool,
            tc.tile_pool(name="rows", bufs=5) as rpool,
            tc.tile_pool(name="sel", bufs=6) as spool,
            tc.tile_pool(name="work", bufs=2) as wpool,
            tc.tile_pool(name="hsec", bufs=3) as hpool,
            tc.tile_pool(name="psum", bufs=2, space="PSUM") as pp,
        ):
            rel_t = pool.tile([P, TOT2], bf16)
            nc.sync.dma_start(out=rel_t[:], in_=rel[:])
            iota_t = pool.tile([P, KMAX * HD], bf16)
            nc.sync.dma_start(out=iota_t[:], in_=iota[:])
            ex_t = pool.tile([P, NBA], f32)
            nc.sync.dma_start(out=ex_t[:], in_=exsh[:])
            s2p_t = pool.tile([P, NC], f32)
            nc.sync.dma_start(out=s2p_t[:], in_=s2p[:])

            s2tot = pool.tile([P, 1], f32)
            nc.vector.tensor_reduce(
                out=s2tot[:], in_=s2p_t[:], axis=mybir.AxisListType.X, op=Alu.add
            )
            rs2c = pool.tile([P, 1], f32)
            nc.vector.reciprocal(out=rs2c[:], in_=s2tot[:])

            hsec_t = None
            hs = 0
            off = 0
            for blks in groups:
                gtot = sum(nblk[b] for b in blks)
                rt = rpool.tile([P, GMAX * D], bf16, tag="rows")
                nc.sync.dma_start(
                    out=rt[:, : gtot * D], in_=rows[:, off * D : (off + gtot) * D]
                )
                loc = 0
                for b in blks:
                    nb0, nb1 = int(cb2[2 * b]), int(cb2[2 * b + 1])
                    if b == HSEC[hs]:
                        hsec_t = hpool.tile(
                            [P, (HSEC[hs + 1] - HSEC[hs]) * D], bf16, tag="hsec"
                        )
                    ps = pp.tile([P, D], f32, tag="ps", space="PSUM")
                    for half, h0, nbh in ((0, 0, nb0), (1, nb0, nb1)):
                        ci = 0
                        for g0 in range(0, nbh, KMAX):
                            G = min(KMAX, nbh - g0)
                            sel = spool.tile([P, KMAX * HD], bf16, tag="sel")
                            nc.vector.tensor_tensor(
                                out=sel[:, : G * HD],
                                in0=iota_t[:, : G * HD],
                                in1=rel_t[
                                    :, off + loc + h0 + g0 : off + loc + h0 + g0 + G
                                ].to_broadcast([P, G, HD]),
                                op=Alu.is_equal,
                            )
                            for j in range(G):
                                c = loc + h0 + g0 + j
                                nc.tensor.matmul(
                                    out=ps[half * HD : (half + 1) * HD, :],
                                    lhsT=sel[:, j * HD : (j + 1) * HD],
                                    rhs=rt[:, c * D : (c + 1) * D],
                                    start=(ci == 0), stop=(ci == nbh - 1),
                                )
                                ci += 1
                    vcol = wpool.tile([P, 1], f32, tag="vcol")
                    nc.vector.tensor_tensor(
                        out=vcol[:], in0=ex_t[:, b : b + 1], in1=rs2c[:], op=Alu.mult
                    )
                    ho = (b - HSEC[hs]) * D
                    nc.scalar.activation(
                        out=hsec_t[:, ho : ho + D], in_=ps[:], func=Act.Copy,
                        scale=vcol[:],
                    )
                    if b == HSEC[hs + 1] - 1:
                        nc.sync.dma_start(
                            out=hno[:, HSEC[hs] * D : HSEC[hs + 1] * D], in_=hsec_t[:]
                        )
                        hs += 1
                    loc += nb0 + nb1
                off += gtot
    return nc


# ---------------------------------------------------------------- host glue
def _route(key, ngroups, nblocks, payload_idx, slot):
    """Sort incidences by (core, block-half) key; pad each (block, half) to
    core-uniform chunk counts. Returns (per-core payload-index array [TOT*P],
    per-core rel array [TOT*P] (-1 = pad), cb [nblocks] chunks per group)."""
    cnt = np.bincount(key, minlength=ngroups)
    cb = np.maximum(1, -(-cnt.reshape(NC, nblocks) // P)).max(axis=0)  # ceil
    chunkbase = np.zeros(nblocks, np.int64)
    np.cumsum(cb[:-1], out=chunkbase[1:])
    TOT = int(cb.sum())
    order = np.argsort(key, kind="stable")
    ks = key[order]
    gstart = np.zeros(ngroups, np.int64)
    np.cumsum(cnt[:-1], out=gstart[1:])
    rank = np.arange(M, dtype=np.int64) - gstart[ks]
    b_s = ks % nblocks
    c_s = ks // nblocks
    pos = chunkbase[b_s] * P + rank
    pidx = np.zeros((NC, TOT * P), np.int64)
    relv = np.full((NC, TOT * P), -1.0, np.float32)
    pidx[c_s, pos] = payload_idx[order]
    relv[c_s, pos] = slot[order]
    return pidx, relv, cb


def _pack_rows(table, pidx, relv, ew):
    """Gather rows per incidence slot and lay out partition-major:
    out[p, ci*ew:(ci+1)*ew] = table[pidx[ci*P+p]]."""
    TOTP = pidx.shape[0]
    r = table[pidx]  # [TOT*P, ew]
    r = np.ascontiguousarray(
        r.reshape(TOTP // P, P, ew).transpose(1, 0, 2).reshape(P, TOTP // P * ew)
    )
    rl = np.ascontiguousarray(relv.reshape(TOTP // P, P).T.astype(BF))
    return r, rl


def kernel(x, W, b, attn_node, attn_edge, node_idx, he_idx, num_hyperedges):
    x = np.asarray(x, np.float32)
    W = np.asarray(W, np.float32)
    b = np.asarray(b, np.float32).reshape(-1)
    attn_node = np.asarray(attn_node, np.float32).reshape(-1)
    attn_edge = np.asarray(attn_edge, np.float32).reshape(-1)
    node_idx = np.asarray(node_idx).astype(np.int64)
    he_idx = np.asarray(he_idx).astype(np.int64)
    assert x.shape == (N, D) and node_idx.shape == (M,) and int(num_hyperedges) == H
    LAST_EXEC_TIMES.clear()

    iota_np = np.ascontiguousarray(
        np.tile(np.arange(P // 2, dtype=np.float32), (P, KMAX)).astype(BF)
    )
    ones_row = np.ones((1, P), BF)
    ones_col = np.ones((P, 1), BF)

    # ---------- launch A ----------
    nc_a = _build_launch_a()
    xT = np.ascontiguousarray(x.T)  # [128, N]
    cnt = np.bincount(node_idx, minlength=N).astype(np.float32)
    ins_a = []
    for c in range(NC):
        xts = np.zeros((P, NSHP), np.float32)
        xts[:, :NSH] = xT[:, c * NSH : (c + 1) * NSH]
        cnt_p = np.zeros(NSHP, np.float32)
        cnt_p[:NSH] = cnt[c * NSH : (c + 1) * NSH]
        ins_a.append(
            {
                "xT": xts,
                "W": W.astype(BF),
                "WT": np.ascontiguousarray(W.T).astype(BF),
                "b_col": b.reshape(D, 1).astype(BF),
                "an_col": attn_node.reshape(D, 1).astype(BF),
                "ones_row": ones_row,
                "ones_col": ones_col,
                "cnt_w": np.ascontiguousarray(cnt_p.reshape(NBA, P).T),
            }
        )
    res_a = _run(nc_a, ins_a, "A")
    g_full = np.concatenate(
        [
            np.asarray(res_a[c]["g_sh"])
            .reshape(P, NBA, EW)
            .transpose(1, 0, 2)
            .reshape(NSHP, EW)[:NSH]
            for c in range(NC)
        ],
        axis=0,
    )  # [N, EW] bf16
    s1p_np = np.concatenate(
        [np.asarray(res_a[c]["s1_part"]) for c in range(NC)], axis=1
    )  # [1, 8] f32

    # ---------- stage-1 routing (host halo exchange) ----------
    c1 = he_idx // HSH
    b1 = (he_idx % HSH) // P
    s1v = (he_idx % HSH) % P
    hf1 = s1v // (P // 2)
    slot1 = (s1v % (P // 2)).astype(np.float32)
    key1 = ((c1 * NBB + b1) * 2 + hf1).astype(np.int64)
    pidx1, relv1, cb1 = _route(key1, NC * NBB * 2, NBB * 2, node_idx, slot1)

    nc_b = _build_launch_b(cb1.tolist())
    ae_bc = np.tile(attn_edge.reshape(1, D), (P, 1)).astype(np.float32)
    b_bc = np.tile(b.reshape(1, D), (P, 1)).astype(np.float32)
    ins_b = []
    for c in range(NC):
        r, rl = _pack_rows(g_full, pidx1[c], relv1[c], EW)
        ins_b.append(
            {
                "rows": r,
                "rel": rl,
                "iota": iota_np,
                "ae_bc": ae_bc,
                "b_bc": b_bc,
                "s1p": np.ascontiguousarray(np.tile(s1p_np, (P, 1))),
                "ones_col": ones_col,
            }
        )
    res_b = _run(nc_b, ins_b, "B")
    t2_full = np.concatenate(
        [
            np.asarray(res_b[c]["t2o"])
            .reshape(P, NBB, D)
            .transpose(1, 0, 2)
            .reshape(NBB * P, D)[:HSH]
            for c in range(NC)
        ],
        axis=0,
    )  # [H, D] bf16
    s2p_np = np.concatenate(
        [np.asarray(res_b[c]["s2_part"]) for c in range(NC)], axis=1
    )  # [1, 8] f32

    # ---------- stage-2 routing ----------
    c2 = node_idx // NSH
    b2 = (node_idx % NSH) // P
    s2v = (node_idx % NSH) % P
    hf2 = s2v // (P // 2)
    slot2 = (s2v % (P // 2)).astype(np.float32)
    key2 = ((c2 * NBA + b2) * 2 + hf2).astype(np.int64)
    pidx2, relv2, cb2 = _route(key2, NC * NBA * 2, NBA * 2, he_idx, slot2)

    nc_c = _build_launch_c(cb2.tolist())
    ins_c = []
    for c in range(NC):
        r, rl = _pack_rows(t2_full, pidx2[c], relv2[c], D)
        ins_c.append(
            {
                "rows": r,
                "rel": rl,
                "iota": iota_np,
                "exsh": np.asarray(res_a[c]["exan_sh"]),
                "s2p": np.ascontiguousarray(np.tile(s2p_np, (P, 1))),
            }
        )
    res_c = _run(nc_c, ins_c, "C")
    h_n = np.concatenate(
        [
            np.asarray(res_c[c]["hno"])
            .reshape(P, NBA, D)
            .transpose(1, 0, 2)
            .reshape(NSHP, D)[:NSH]
            .astype(np.float32)
            for c in range(NC)
        ],
        axis=0,
    )
    return h_n
